# revision 1
# baseline (speedup 1.0000x reference)
"""LSH decoder kernel for Trainium2 (8 NeuronCores, Bass/Tile).

Problem: N=8192 points, D=256. Output[i,m] = 1.0 iff
  (i != m) AND cosine(Z_i, Z_m) > 0.5 AND the two points share an LSH
  band bucket (some band's 8 hyperplane signs identical).

Strategy
--------
The cosine gate is the binding constraint: the output can only be nonzero
where cos > 0.5. The kernel computes, per core, a [1024, 8192] slab of
relu(cos - 0.49) (exact zeros below threshold) plus a scalar flag =
sum(slab). Whenever the reference output has ANY nonzero pair, that pair
has cos > 0.5 > 0.49 + (bf16 matmul error bound), so the flag is
guaranteed nonzero. If every core's flag is exactly 0.0, all off-diagonal
cosines are <= 0.49 + eps < 0.5, hence the reference output is identically
zero and so is ours -- exact. If a flag fires (never, for gaussian data),
the host recomputes the full reference semantics (including the per-band
signature match) in fp32 NumPy.

SPMD trick: every core runs the same program; core k receives
np.roll(Z, -k*1024, axis=0) so its own 1024 rows sit at local columns
0..1023. That makes the self-pair (diagonal) block position static --
it is masked in PSUM before thresholding. The host un-rotates each slab
with np.roll when assembling the full [8192, 8192] output.
"""

import sys

import numpy as np

if "/opt/trn_rl_repo" not in sys.path:
    sys.path.insert(0, "/opt/trn_rl_repo")

N = 8192
D = 256
N_CORES = 8
SLAB = N // N_CORES  # 1024 rows per core
BANDS = 16
ROWS = 8
SIM_THRESH = 0.5
FLAG_THRESH = 0.49  # 0.5 minus a safety margin >> bf16 matmul error bound
EPS = 1e-8

_CACHE = {}


def _build_nc():
    import concourse.bass as bass
    import concourse.mybir as mybir
    import concourse.tile as tile
    from concourse import bacc
    from concourse.masks import make_identity

    f32 = mybir.dt.float32
    bf16 = mybir.dt.bfloat16

    nc = bacc.Bacc(
        "TRN2",
        target_bir_lowering=False,
        debug=False,
        enable_asserts=False,
        num_devices=N_CORES,
    )

    # zn: row-normalized Z (exact fp32 on host, cast bf16), rotated per core
    zn_dram = nc.dram_tensor("zn", [N, D], bf16, kind="ExternalInput").ap()
    out_dram = nc.dram_tensor("out", [SLAB, N], f32, kind="ExternalOutput").ap()
    flag_dram = nc.dram_tensor("flag", [1, 128], f32, kind="ExternalOutput").ap()

    NT = N // 128  # 64 row tiles of Z
    IT = SLAB // 128  # 8 output row tiles
    NBLK = 4  # Znt column blocks of 2048
    BLKW = N // NBLK  # 2048
    CHUNK = 512  # matmul free dim (one PSUM bank)
    CPB = BLKW // CHUNK  # 4 chunks per block

    from contextlib import ExitStack

    with tile.TileContext(nc) as tc, ExitStack() as ctx:
        const_pool = ctx.enter_context(tc.tile_pool(name="const", bufs=1))
        znb_pool = ctx.enter_context(tc.tile_pool(name="znb", bufs=6))
        tp_pool = ctx.enter_context(tc.tile_pool(name="tp", bufs=2, space="PSUM"))
        ps_pool = ctx.enter_context(tc.tile_pool(name="ps", bufs=5, space="PSUM"))
        pf_pool = ctx.enter_context(tc.tile_pool(name="pf", bufs=1, space="PSUM"))
        out_pool = ctx.enter_context(tc.tile_pool(name="out", bufs=4))

        # Constants
        ident = const_pool.tile([128, 128], bf16)
        make_identity(nc, ident[:])
        ome = const_pool.tile([128, 128], f32)  # 1 - I (diagonal mask)
        nc.gpsimd.memset(ome[:], 1.0)
        nc.gpsimd.affine_select(
            out=ome[:],
            in_=ome[:],
            compare_op=mybir.AluOpType.not_equal,
            fill=0.0,
            base=0,
            pattern=[[-1, 128]],
            channel_multiplier=1,
        )
        ones = const_pool.tile([128, 1], f32)
        nc.gpsimd.memset(ones[:], 1.0)
        nthr = const_pool.tile([128, 1], f32)  # relu bias = -FLAG_THRESH
        nc.gpsimd.memset(nthr[:], -FLAG_THRESH)
        acc = const_pool.tile([128, IT * NBLK * CPB], f32)  # flag accumulators

        # Normalized, transposed Z in bf16: 2 d-halves x 4 column blocks
        znt = [
            [
                const_pool.tile([128, BLKW], bf16, name=f"znt_{h}_{b}")
                for b in range(NBLK)
            ]
            for h in range(2)
        ]

        # Phase 1: load pre-normalized bf16 rows, transpose via PE
        for t in range(NT):
            znb = znb_pool.tile([128, D], bf16)
            nc.sync.dma_start(znb[:], zn_dram[t * 128 : (t + 1) * 128, :])

            blk, off = t // 16, (t % 16) * 128
            for h in range(2):
                tp = tp_pool.tile([128, 128], bf16)
                nc.tensor.transpose(tp[:], znb[:, h * 128 : (h + 1) * 128], ident[:])
                nc.vector.tensor_copy(znt[h][blk][:, off : off + 128], tp[:])

        # Phase 2: cosine slab, threshold, flag accumulation, store
        for it in range(IT):
            lhs = [znt[h][0][:, it * 128 : (it + 1) * 128] for h in range(2)]
            for cg in range(NBLK):
                ot = out_pool.tile([128, BLKW], f32)
                pss = []
                for cc in range(CPB):
                    ps = ps_pool.tile([128, CHUNK], f32)
                    pss.append(ps)
                    nc.tensor.matmul(
                        ps[:],
                        lhs[0],
                        znt[0][cg][:, cc * CHUNK : (cc + 1) * CHUNK],
                        start=True,
                        stop=False,
                    )
                for cc in range(CPB):
                    nc.tensor.matmul(
                        pss[cc][:],
                        lhs[1],
                        znt[1][cg][:, cc * CHUNK : (cc + 1) * CHUNK],
                        start=False,
                        stop=True,
                    )
                for cc in range(CPB):
                    ps = pss[cc]
                    # Self-pair (diagonal) block: rows it*128+p pair with local
                    # column it*128+p, always inside column group 0.
                    if cg == 0 and cc == it * 128 // CHUNK:
                        o = it * 128 % CHUNK
                        nc.vector.tensor_mul(
                            ps[:, o : o + 128], ps[:, o : o + 128], ome[:]
                        )
                    idx = (it * NBLK + cg) * CPB + cc
                    nc.scalar.activation(
                        ot[:, cc * CHUNK : (cc + 1) * CHUNK],
                        ps[:],
                        mybir.ActivationFunctionType.Relu,
                        bias=nthr[:],
                        scale=1.0,
                        accum_out=acc[:, idx : idx + 1],
                    )
                nc.sync.dma_start(
                    out_dram[it * 128 : (it + 1) * 128, cg * BLKW : (cg + 1) * BLKW],
                    ot[:],
                )

        # Phase 3: flag[1,128] = per-column partition-sums of acc via ones-matmul
        # (host sums the 128 values; >0 iff any relu output anywhere was >0)
        psf = pf_pool.tile([1, 128], f32)
        nc.tensor.matmul(psf[:], ones[:], acc[:], start=True, stop=True)
        fsb = const_pool.tile([1, 128], f32)
        nc.scalar.copy(fsb[:], psf[:])
        nc.sync.dma_start(flag_dram[:, :], fsb[:])

    nc.compile()
    return nc


def _get_nc():
    if "nc" not in _CACHE:
        _CACHE["nc"] = _build_nc()
    return _CACHE["nc"]


def _exact_fallback(Z, planes):
    """Full fp32 reference semantics on the host (runs only if a flag fires)."""
    Zf = Z.astype(np.float32)
    proj = planes.astype(np.float32) @ Zf.T  # [BANDS*ROWS, N]
    sig = ((proj >= 0).astype(np.float32) * 2.0 - 1.0).reshape(N, BANDS, ROWS)
    same = np.zeros((N, N), dtype=bool)
    for b in range(BANDS):
        s = np.ascontiguousarray(sig[:, b, :])  # [N, ROWS]
        same |= (s @ s.T) == float(ROWS)
    norms = np.maximum(np.linalg.norm(Zf, axis=1), EPS)
    cos = (Zf @ Zf.T) / (norms[:, None] * norms[None, :])
    np.fill_diagonal(same, False)
    return (same & (cos > SIM_THRESH)).astype(np.float32)


def kernel(Z, planes):
    import ml_dtypes

    from concourse.bass_utils import run_bass_kernel_spmd

    Z = np.ascontiguousarray(np.asarray(Z, dtype=np.float32))
    planes = np.ascontiguousarray(np.asarray(planes, dtype=np.float32))
    assert Z.shape == (N, D) and planes.shape == (BANDS * ROWS, D)

    nc = _get_nc()
    inv = 1.0 / np.maximum(np.linalg.norm(Z, axis=1, keepdims=True), EPS)
    zn = (Z * inv).astype(ml_dtypes.bfloat16)
    in_maps = [
        {"zn": np.ascontiguousarray(np.roll(zn, -k * SLAB, axis=0))}
        for k in range(N_CORES)
    ]
    res = run_bass_kernel_spmd(nc, in_maps, core_ids=list(range(N_CORES)))

    if any(float(r["flag"].sum()) > 0.0 for r in res.results):
        return _exact_fallback(Z, planes)

    return np.concatenate(
        [np.roll(res.results[k]["out"], k * SLAB, axis=1) for k in range(N_CORES)],
        axis=0,
    )



# revision 2
# speedup vs baseline: 5.1726x; 5.1726x over previous
"""LSH decoder kernel for Trainium2 (8 NeuronCores, Bass/Tile).

Problem: N=8192 points, D=256. Output[i,m] = 1.0 iff
  (i != m) AND cosine(Z_i, Z_m) > 0.5 AND the two points share an LSH
  band bucket (some band's 8 hyperplane signs identical).

Strategy
--------
The cosine gate is the binding constraint: output can only be nonzero
where cos > 0.5. The device computes a certificate that NO off-diagonal
pair has cos > TAU (= 0.42) in fp8; combined with a rigorously computed
per-input quantization error bound B (host-side, exact), TAU + B < 0.5
implies the reference output is identically zero. If any flag fires (or
the bound check fails), the host recomputes the full reference
semantics exactly.

Device decomposition (uniform SPMD, core k sees data rolled by k*1024):
  - host ships zt[p, h, c] = fp8(zn^T)[h*128+p, (k*1024+c) % 8192],
    c in [0, 4992): normalized-Z transpose, split into two K-halves for
    single-instruction DoubleRow fp8 matmuls (full K=256 contraction,
    0.5 cycles/row on the PE).
  - for each 128-row tile rt, compute cos against the sliding window of
    4096 columns starting at the tile's own diagonal: PSUM mega-tiles
    [128, 1024]. Row p of tile rt covers pair offsets (0, 4096 - p).
  - self-pairs are killed in PSUM by accumulating -3*I into the
    diagonal 128x128 block (cos_ii - 3 ~ -2 < 0 < TAU).
  - detection alternates between ACT (relu(x - TAU) with free running
    accum_out) and DVE (max-reduce) so both engines stream PSUM in
    parallel. Output is a single [128, 32] flag strip per core.
Host covers the sliver the window misses (pair offsets in
[4096 - p, 4096], ~0.5M pairs) with 64 batched 128x128 sgemms, exact
fp32. Union of device window + host sliver = all N(N-1)/2 pairs.
"""

import sys

import numpy as np

if "/opt/trn_rl_repo" not in sys.path:
    sys.path.insert(0, "/opt/trn_rl_repo")

N = 8192
D = 256
N_CORES = 8
SLAB = N // N_CORES  # 1024 rows per core
RT = SLAB // 128  # 8 row tiles per core
WIN = 4096  # per-row-tile detection window (cols), 4 mega-tiles
MEGA = 1024  # PSUM mega-tile width (2 banks)
TPR = WIN // MEGA  # 4 mega-tiles per row tile
NTILES = RT * TPR  # 32 detect tiles per core
WCOLS = (RT - 1) * 128 + WIN  # 4992 input columns per core
BANDS = 16
ROWS = 8
SIM_THRESH = 0.5
TAU = 0.42  # device flag threshold on fp8 cosines
EPS = 1e-8

_CACHE = {}


def _build_nc():
    import concourse.bass as bass
    import concourse.mybir as mybir
    import concourse.tile as tile
    from concourse import bacc
    from concourse.masks import make_identity

    f32 = mybir.dt.float32
    bf16 = mybir.dt.bfloat16
    fp8 = mybir.dt.float8e4
    DR = mybir.MatmulPerfMode.DoubleRow

    nc = bacc.Bacc(
        "TRN2",
        target_bir_lowering=False,
        debug=False,
        enable_asserts=False,
        num_devices=N_CORES,
    )

    zt_dram = nc.dram_tensor("zt", [128, 2, WCOLS], fp8, kind="ExternalInput").ap()
    flag_dram = nc.dram_tensor("flag", [128, NTILES], f32, kind="ExternalOutput").ap()

    from contextlib import ExitStack

    with tile.TileContext(nc) as tc, ExitStack() as ctx:
        const_pool = ctx.enter_context(tc.tile_pool(name="const", bufs=1))
        in_pool = ctx.enter_context(tc.tile_pool(name="in", bufs=1))
        ps_pool = ctx.enter_context(tc.tile_pool(name="ps", bufs=4, space="PSUM"))
        scrap_pool = ctx.enter_context(tc.tile_pool(name="scrap", bufs=2))

        izt = in_pool.tile([128, 2, WCOLS], fp8)
        # Column-sliced input DMAs so compute starts as soon as the first
        # window's worth of columns lands.
        SLICES = [0, 1024, 2048, 3072, 4096, WCOLS]
        for s in range(len(SLICES) - 1):
            a, b = SLICES[s], SLICES[s + 1]
            nc.sync.dma_start(izt[:, :, a:b], zt_dram[:, :, a:b])

        ident = const_pool.tile([128, 128], fp8)
        make_identity(nc, ident[:])
        neg3 = const_pool.tile([128, 128], fp8)
        nc.vector.tensor_scalar_mul(neg3[:], ident[:], -3.0)
        nthr = const_pool.tile([128, 1], f32)
        nc.gpsimd.memset(nthr[:], -TAU)
        strip = const_pool.tile([128, NTILES], f32)

        for rt in range(RT):
            lhsT = izt[:, :, rt * 128 : rt * 128 + 128]
            for t in range(TPR):
                g = rt * TPR + t
                base = rt * 128 + t * MEGA
                mt = ps_pool.tile([128, MEGA], f32)
                for c in range(MEGA // 512):
                    lo = c * 512
                    if t == 0 and c == 0:
                        # diag block lives at cols [0, 128) of this chunk
                        nc.tensor.matmul(
                            mt[:, lo : lo + 512],
                            lhsT,
                            izt[:, :, base + lo : base + lo + 512],
                            start=True,
                            stop=False,
                            perf_mode=DR,
                        )
                        nc.tensor.matmul(
                            mt[:, 0:128], neg3[:], ident[:], start=False, stop=True
                        )
                    else:
                        nc.tensor.matmul(
                            mt[:, lo : lo + 512],
                            lhsT,
                            izt[:, :, base + lo : base + lo + 512],
                            start=True,
                            stop=True,
                            perf_mode=DR,
                        )
                if g % 2 == 0:
                    scrap = scrap_pool.tile([128, MEGA], bf16)
                    nc.scalar.activation(
                        scrap[:, :],
                        mt[:, :],
                        mybir.ActivationFunctionType.Relu,
                        bias=nthr[:],
                        scale=1.0,
                        accum_out=strip[:, g : g + 1],
                    )
                else:
                    nc.vector.tensor_reduce(
                        strip[:, g : g + 1],
                        mt[:, :],
                        axis=mybir.AxisListType.X,
                        op=mybir.AluOpType.max,
                    )

        nc.sync.dma_start(flag_dram[:, :], strip[:])

    nc.compile()
    return nc


def _get_nc():
    if "nc" not in _CACHE:
        _CACHE["nc"] = _build_nc()
    return _CACHE["nc"]


def _exact_fallback(Z, planes):
    """Full fp32 reference semantics on the host (safety net only)."""
    Zf = Z.astype(np.float32)
    proj = planes.astype(np.float32) @ Zf.T  # [BANDS*ROWS, N]
    sig = ((proj >= 0).astype(np.float32) * 2.0 - 1.0).reshape(N, BANDS, ROWS)
    same = np.zeros((N, N), dtype=bool)
    for b in range(BANDS):
        s = np.ascontiguousarray(sig[:, b, :])  # [N, ROWS]
        same |= (s @ s.T) == float(ROWS)
    norms = np.maximum(np.linalg.norm(Zf, axis=1), EPS)
    cos = (Zf @ Zf.T) / (norms[:, None] * norms[None, :])
    np.fill_diagonal(same, False)
    return (same & (cos > SIM_THRESH)).astype(np.float32)


def kernel(Z, planes):
    import ml_dtypes

    from concourse.bass_utils import run_bass_kernel_spmd

    Z = np.ascontiguousarray(np.asarray(Z, dtype=np.float32))
    planes = np.ascontiguousarray(np.asarray(planes, dtype=np.float32))
    assert Z.shape == (N, D) and planes.shape == (BANDS * ROWS, D)

    inv = 1.0 / np.maximum(np.linalg.norm(Z, axis=1, keepdims=True), EPS)
    zn = Z * inv
    q8 = zn.astype(ml_dtypes.float8_e4m3fn)
    qf = q8.astype(np.float32)

    # Rigorous per-input bound on |fp8_cos - cos| (PE accumulates fp32):
    # chat - c = -e_i.z_j - z_i.e_j + e_i.e_j with e = zn - fp8(zn).
    e = zn - qf
    emax = float(np.linalg.norm(e, axis=1).max())
    zmax = float(np.linalg.norm(zn, axis=1).max())
    bound = 2.0 * emax * zmax + emax * emax + 1e-3
    if TAU + bound >= SIM_THRESH:
        return _exact_fallback(Z, planes)

    # Host covers the sliver the device window misses: pairs (i, i+d) with
    # d in [4096 - (i % 128), 4096]. For block t (rows 128t+p), partner
    # cols are 128t + 4096 + q with q <= p: lower triangle of 64 batched
    # 128x128 gram blocks. Exact fp32 -> compare against SIM_THRESH.
    zb = zn.reshape(N // 128, 128, D)
    zr = np.roll(zn, -WIN, axis=0).reshape(N // 128, 128, D)
    G = np.matmul(zb, zr.transpose(0, 2, 1))  # [64, 128, 128]
    tril = np.tril(np.ones((128, 128), dtype=bool))
    if float(G[:, tril].max()) > SIM_THRESH:
        return _exact_fallback(Z, planes)

    # Per-core rolled, transposed, K-half-split fp8 input.
    q8T = np.ascontiguousarray(q8.T)  # [256, N] fp8
    in_maps = []
    for k in range(N_CORES):
        blk = np.roll(q8T, -k * SLAB, axis=1)[:, :WCOLS]  # [256, WCOLS]
        zt = np.ascontiguousarray(blk.reshape(2, 128, WCOLS).transpose(1, 0, 2))
        in_maps.append({"zt": zt})

    nc = _get_nc()
    res = run_bass_kernel_spmd(nc, in_maps, core_ids=list(range(N_CORES)))

    for r in res.results:
        strip = np.asarray(r["flag"], dtype=np.float32)  # [128, NTILES]
        acc = strip[:, 0::2]  # ACT relu-sum columns
        mx = strip[:, 1::2]  # DVE max columns
        if float(acc.sum()) > 0.0 or float(mx.max()) > TAU:
            return _exact_fallback(Z, planes)

    return np.zeros((N, N), dtype=np.float32)
